# revision 34
# baseline (speedup 1.0000x reference)
"""Trainium2 Bass kernel for nn_CentroidModel (retrieval_knn).

Computes out = -(||e||^2 + ||c||^2 - 2 e.c) with e = x @ W, over 8
NeuronCores, data-parallel on the batch dim (x rows sharded; W and
centroids replicated).

All GEMMs run in fp8e4 (e4m3) with MatmulPerfMode.DoubleRow: operands
packed [128, 2, free] so each matmul contracts 256 rows.  Measured on
HW: the matmul streams at 0.5 cyc/out-row but every matmul pays a
serial ~134 ns LDWEIGHTS (256-col stationary), which sets the PE floor.
The error budget (rel 2e-2 of an output dominated by ||e||^2+||c||^2
~ 1100) is ~28 absolute; fp8 quantization costs ~12, fp16 output ~1.

Engine split per 2048-centroid chunk (8 chunks):
  PE:  24 DoubleRow matmuls (cross) + 4 ones-matmuls (c_sq reduce)
  ACT: psum evac w/ -e_sq bias -> fp16 for half the i-blocks, negated
       c_sq evacs, and the ct/ct2 load issues (ACT HWDGE ring)
  DVE: fused (psum - e_sq) - c_sq scalar_tensor_tensor for the other
       i-blocks, bf16 squares/adds for c_sq (4x mode), fp16 adds
  GpSimd: nothing — measured ~6x slower than DVE on bulk elementwise
  SP:  output stores + input loads (SP HWDGE ring)

The bf16 ct copy exists because DVE's fp8-input path measured ~5
cycles/element while the all-16-bit path hits the 4x mode; it also
makes c_sq more accurate (no fp8^2 error).  e_sq comes from ACT Square
directly off the phase-1 psum; its ONES-matmul partition-reduce is
emitted after the whole phase-1 loop so the in-order PE queue never
waits on ACT.
"""

import numpy as np

_B, _DIN, _D, _C = 8192, 1024, 768, 16384
_NCORES = 8
_B_LOC = _B // _NCORES

_P = 128  # SBUF/PSUM partitions
_NT = 512  # matmul free-dim tile (one f32 PSUM bank)
_NW = 2048  # main-loop working width (c-chunk)
_NP = 1024  # psum tile width (two banks)

# i-blocks whose psum evac+fixup runs fused on DVE (rest go ACT+add);
# measured: ACT evac ~1.0 ns/col, DVE STT ~1.15, DVE 16-bit ~0.5, so ACT
# takes most of the evacs and GpSimd absorbs two non-critical adds.
_DVE_I = (3, 6)
_GPS_I = (1, 5)


def emit_centroid_kernel(tc, xt, w, ct, ct2, out, b_loc, din, d, c):
    """Emit the per-core Tile kernel.

    xt:  [(jx,p), 2, b_loc] fp8e4    (x shard, DoubleRow-packed)
    w:   [(jx,p), 2, d] fp8e4        (64*W, DoubleRow-packed)
    ct:  [(jd,p), 2, c] fp8e4        (centroids.T, DoubleRow-packed)
    ct2: [(jd,p), 2, c] bf16         (same layout, feeds the c_sq squares)
    out: [b_loc, c] fp16
    """
    from concourse import mybir
    from concourse.alu_op_type import AluOpType
    from concourse.masks import make_identity

    nc = tc.nc
    e4 = mybir.dt.float8e4
    bf16 = mybir.dt.bfloat16
    f16 = mybir.dt.float16
    f32 = mybir.dt.float32
    AF = mybir.ActivationFunctionType
    DR = mybir.MatmulPerfMode.DoubleRow

    jx = din // (2 * _P)  # k-pairs over d_in
    jd = d // (2 * _P)  # k-pairs over d (embedding)
    md = d // _P  # 128-blocks over d
    mb = b_loc // _P  # tiles over local batch
    npair = c // _NW  # c-chunks

    with (
        tc.tile_pool(name="persist", bufs=1) as persist,
        tc.tile_pool(name="ct_in", bufs=9) as ct_pool,
        tc.tile_pool(name="ct2_in", bufs=6) as ct2_pool,
        tc.tile_pool(name="sq", bufs=3) as sq_pool,
        tc.tile_pool(name="csum", bufs=2) as csum_pool,
        tc.tile_pool(name="csqs", bufs=3) as csq_pool,
        tc.tile_pool(name="t1", bufs=3) as t1_pool,
        tc.tile_pool(name="outs", bufs=5) as out_pool,
        tc.tile_pool(name="scratch", bufs=2) as scratch,
    ):
        # ---- persistent SBUF tensors ----
        xt_s = [persist.tile([_P, 2, b_loc], e4, name=f"xt{j}", tag=f"xt{j}") for j in range(jx)]
        w_s = [persist.tile([_P, 2, d], e4, name=f"w{j}", tag=f"w{j}") for j in range(jx)]
        et2_s = [persist.tile([_P, 2, b_loc], e4, name=f"et{j}", tag=f"et{j}") for j in range(jd)]
        negesq = persist.tile([_P, mb], f32, name="negesq", tag="negesq")
        ones = persist.tile([_P, _P], bf16, name="ones", tag="ones")
        ident = persist.tile([_P, _P], f32, name="ident", tag="ident")

        for j in range(jx):
            nc.sync.dma_start(w_s[j][:], w[j * _P : (j + 1) * _P, :, :])
            nc.sync.dma_start(xt_s[j][:], xt[j * _P : (j + 1) * _P, :, :])
        # emitted after the input loads: only needed by e_sq/c_sq, and
        # emitting them first would delay the PE-critical xt/w arrival
        nc.vector.memset(ones[:], 1.0)
        make_identity(nc, ident[:])

        def load_ct(n):
            # both centroid copies ride the ACT HWDGE ring, leaving the
            # SP ring to the (larger) output stores
            csl = slice(n * _NW, (n + 1) * _NW)
            tiles = []
            for j in range(jd):
                t = ct_pool.tile([_P, 2, _NW], e4, name=f"ct{j}", tag="ct")
                nc.scalar.dma_start(t[:], ct[j * _P : (j + 1) * _P, :, csl])
                tiles.append(t)
            tiles2 = []
            for j in range(jd):
                t2 = ct2_pool.tile([_P, 2, _NW], bf16, name=f"cu{j}", tag="cu")
                nc.scalar.dma_start(t2[:], ct2[j * _P : (j + 1) * _P, :, csl])
                tiles2.append(t2)
            return tiles, tiles2

        # the first two chunks' centroid loads are issued BEFORE the
        # prologue is emitted: the Scalar engine queue is in-order, so
        # issuing them here (while it is still empty) lets the DMAs run
        # under the prologue instead of queueing behind its psum evacs.
        ct_q = [load_ct(0), load_ct(1)]

        # ---- prologue (own PSUM scope, freed before the main loop) ----
        with tc.tile_pool(name="ps_pro", bufs=2, space="PSUM") as ps_pro:
            # phase 1: et2 = fp8(2 * (W.T @ xT));  psum = 64 * eT.
            # sqe = (2e)^2 via ACT Square off the same psum.
            sqes = []
            for m in range(md):
                pt = ps_pro.tile([_P, b_loc], f32, name="pro", tag="pro")
                for nb in range(b_loc // _NT):
                    bs = slice(nb * _NT, (nb + 1) * _NT)
                    for j in range(jx):
                        nc.tensor.matmul(
                            pt[:, bs],
                            w_s[j][:, :, m * _P : (m + 1) * _P],
                            xt_s[j][:, :, bs],
                            start=(j == 0),
                            stop=(j == jx - 1),
                            perf_mode=DR,
                        )
                nc.scalar.activation(
                    et2_s[m // 2][:, m % 2, :], pt[:], AF.Copy, scale=0.03125
                )
                sqe = scratch.tile([_P, b_loc], bf16, name="sqe", tag="sqe", bufs=md)
                nc.scalar.activation(sqe[:], pt[:], AF.Square, scale=0.03125)
                sqes.append(sqe)
            # phase 2 (emitted after the whole m-loop so the PE never
            # head-of-line waits on the ACT Squares): ONES-matmul
            # partition-reduce of sum_m sqe -> 4*e_sq replicated.
            pesq = ps_pro.tile([_P, b_loc], f32, name="pesq", tag="pesq", bufs=1)
            for m in range(md):
                for nb in range(b_loc // _NT):
                    bs = slice(nb * _NT, (nb + 1) * _NT)
                    nc.tensor.matmul(
                        pesq[:, bs],
                        ones[:],
                        sqes[m][:, bs],
                        start=(m == 0),
                        stop=(m == md - 1),
                    )
            # PE-transpose each [128, 128] slice so e_sq lands
            # per-partition; sqe held (2e)^2 so the scale is -1/4.
            esq_rep = scratch.tile([_P, b_loc], f32, name="esq_rep", tag="esq_rep")
            nc.scalar.activation(esq_rep[:], pesq[:], AF.Copy)
            for i in range(mb):
                ptr = ps_pro.tile([_P, _P], f32, name="ptr", tag="ptr")
                nc.tensor.transpose(ptr[:], esq_rep[:, i * _P : (i + 1) * _P], ident[:])
                nc.scalar.activation(negesq[:, i : i + 1], ptr[:, 0:1], AF.Copy, scale=-0.25)

        # ---- main loop over c-chunks (2048 centroids each) ----
        with (
            tc.tile_pool(name="ps_big", bufs=3, space="PSUM") as ps_big,
            tc.tile_pool(name="ps_csq", bufs=1, space="PSUM") as ps_csq,
        ):
            def csq_front(ct2_t):
                # sum_k ct^2: all-bf16 DVE squares + adds; the k2-fold goes
                # to GpSimd (slow but off the DVE critical path)
                sqs = []
                for j in range(jd):
                    sq_t = sq_pool.tile([_P, 2, _NW], bf16, name="sqc", tag="sqc")
                    nc.vector.tensor_mul(sq_t[:], ct2_t[j][:], ct2_t[j][:])
                    sqs.append(sq_t)
                nc.vector.tensor_add(sqs[0][:], sqs[0][:], sqs[1][:])
                nc.vector.tensor_add(sqs[0][:], sqs[0][:], sqs[2][:])
                csum = csum_pool.tile([_P, _NW], bf16, name="csum", tag="csum")
                nc.gpsimd.tensor_add(csum[:], sqs[0][:, 0, :], sqs[0][:, 1, :])
                return csum

            def csq_back(csum):
                # ONES-matmul partition-reduce, ACT-evacuated NEGATED to
                # fp16 so the output fixup is an add. The 2-bank psum
                # tile is reused for both 1024-halves.
                csq_s = csq_pool.tile([_P, _NW], f16, name="csq_s", tag="csq_s")
                for g in range(_NW // _NP):
                    pcs = ps_csq.tile([_P, _NP], f32, name="csq", tag="csq")
                    for h in range(2):
                        nc.tensor.matmul(
                            pcs[:, h * _NT : (h + 1) * _NT],
                            ones[:],
                            csum[:, g * _NP + h * _NT : g * _NP + (h + 1) * _NT],
                            start=True,
                            stop=True,
                        )
                    nc.scalar.activation(
                        csq_s[:, g * _NP : (g + 1) * _NP], pcs[:], AF.Copy, scale=-1.0
                    )
                return csq_s

            # csq's DVE work starts as soon as the chunk's ct lands, but its
            # PE matmuls + ACT evac are emitted LATE (at i == mb-2) so they
            # never head-of-line block the in-order PE queue while the DVE
            # chain finishes.  ct is prefetched TWO chunks ahead so loads
            # never arrive late even when the Scalar queue is backed up.
            csq_cur = csq_back(csq_front(ct_q[0][1]))
            for n in range(npair):
                csl = slice(n * _NW, (n + 1) * _NW)
                ct_cur, ct2_cur = ct_q[n % 2]
                if n + 2 < npair:
                    ct_q[n % 2] = load_ct(n + 2)
                csum_nxt = csq_front(ct_q[(n + 1) % 2][1]) if n + 1 < npair else None

                for i in range(mb):
                    # j-outer / seg-inner: the 4 psum segments share the
                    # stationary operand of each j
                    pbs = [
                        ps_big.tile([_P, _NP], f32, name="bigA", tag="big"),
                        ps_big.tile([_P, _NP], f32, name="bigB", tag="big"),
                    ]
                    for j in range(jd):
                        lhsT = et2_s[j][:, :, i * _P : (i + 1) * _P]
                        for s in range(_NW // _NT):
                            nc.tensor.matmul(
                                pbs[s // 2][:, (s % 2) * _NT : (s % 2 + 1) * _NT],
                                lhsT,
                                ct_cur[j][:, :, s * _NT : (s + 1) * _NT],
                                start=(j == 0),
                                stop=(j == jd - 1),
                                perf_mode=DR,
                            )
                    ot = out_pool.tile([_P, _NW], f16, name="ot", tag="ot")
                    if i in _DVE_I:
                        # fused psum evac + e_sq bias + c_sq add on DVE
                        for g in range(2):
                            nc.vector.scalar_tensor_tensor(
                                ot[:, g * _NP : (g + 1) * _NP],
                                pbs[g][:],
                                negesq[:, i : i + 1],
                                csq_cur[:, g * _NP : (g + 1) * _NP],
                                AluOpType.add,
                                AluOpType.add,
                            )
                    else:
                        t1 = t1_pool.tile([_P, _NW], f16, name="t1", tag="t1")
                        for g in range(2):
                            nc.scalar.activation(
                                t1[:, g * _NP : (g + 1) * _NP],
                                pbs[g][:],
                                AF.Identity,
                                bias=negesq[:, i : i + 1],
                            )
                        adder = nc.gpsimd if i in _GPS_I else nc.vector
                        adder.tensor_add(ot[:], t1[:], csq_cur[:])
                    # the last chunk splits its stores across both HWDGE
                    # rings — the loads are done, and it halves the drain
                    # tail after the final compute
                    if n == npair - 1 and i % 2 == 1:
                        nc.scalar.dma_start(out[i * _P : (i + 1) * _P, csl], ot[:])
                    else:
                        nc.sync.dma_start(out[i * _P : (i + 1) * _P, csl], ot[:])
                    if i == mb - 2 and csum_nxt is not None:
                        csq_nxt = csq_back(csum_nxt)
                if csum_nxt is not None:
                    csq_cur = csq_nxt


def build_nc(b_loc=_B_LOC, din=_DIN, d=_D, c=_C):
    import concourse.tile as tile
    from concourse import bacc, mybir

    nc = bacc.Bacc("TRN2", target_bir_lowering=False, debug=False)
    jx = din // (2 * _P)
    jd = d // (2 * _P)
    xt = nc.declare_dram_parameter("xt", [jx * _P, 2, b_loc], mybir.dt.float8e4, isOutput=False)
    w = nc.declare_dram_parameter("w", [jx * _P, 2, d], mybir.dt.float8e4, isOutput=False)
    ct = nc.declare_dram_parameter("ct", [jd * _P, 2, c], mybir.dt.float8e4, isOutput=False)
    ct2 = nc.declare_dram_parameter("ct2", [jd * _P, 2, c], mybir.dt.bfloat16, isOutput=False)
    out = nc.declare_dram_parameter("out", [b_loc, c], mybir.dt.float16, isOutput=True)
    with tile.TileContext(nc) as tc:
        emit_centroid_kernel(tc, xt.ap(), w.ap(), ct.ap(), ct2.ap(), out.ap(), b_loc, din, d, c)
    nc.compile()
    return nc


def _pack_pairs(a2d, dtype):
    """[K, F] -> [(j,p), 2, F] DoubleRow pair layout, row (2j+k2)*128+p."""
    k, f = a2d.shape
    j = k // (2 * _P)
    return np.ascontiguousarray(
        a2d.reshape(j, 2, _P, f).transpose(0, 2, 1, 3).reshape(j * _P, 2, f)
    ).astype(dtype)


def make_in_maps(x, W, centroids, b_loc=_B_LOC, n_cores=_NCORES):
    import ml_dtypes

    e4 = ml_dtypes.float8_e4m3

    x = np.asarray(x, dtype=np.float32)
    W = np.asarray(W, dtype=np.float32)
    centroids = np.asarray(centroids, dtype=np.float32)

    # W is pre-scaled by 64 so its ~0.02-sigma values land in fp8e4's
    # normal range (min normal 2^-6); the kernel rescales by 2/64.
    w_p = _pack_pairs(W * 64.0, e4)  # [(jx,p), 2, D]
    ctt = np.ascontiguousarray(centroids.T)
    ct_p = _pack_pairs(ctt, e4)  # [(jd,p), 2, C]
    ct2_p = _pack_pairs(ctt, ml_dtypes.bfloat16)
    xt_full = np.ascontiguousarray(x.T)  # [DIN, B]

    maps = []
    for i in range(n_cores):
        xt_p = _pack_pairs(xt_full[:, i * b_loc : (i + 1) * b_loc], e4)
        maps.append({"xt": xt_p, "w": w_p, "ct": ct_p, "ct2": ct2_p})
    return maps


_NC_CACHE = {}


def kernel(x, W, centroids):
    from concourse.bass_utils import run_bass_kernel_spmd

    if "nc" not in _NC_CACHE:
        _NC_CACHE["nc"] = build_nc()
    nc = _NC_CACHE["nc"]

    in_maps = make_in_maps(x, W, centroids)
    res = run_bass_kernel_spmd(nc, in_maps, list(range(_NCORES)))
    return np.concatenate(
        [res.results[i]["out"].astype(np.float32) for i in range(_NCORES)], axis=0
    )


# revision 37
# speedup vs baseline: 1.0816x; 1.0816x over previous
"""Trainium2 Bass kernel for nn_CentroidModel (retrieval_knn).

Computes out = -(||e||^2 + ||c||^2 - 2 e.c) with e = x @ W, over 8
NeuronCores, data-parallel on the batch dim (x rows sharded; W and
centroids replicated).

All GEMMs run in fp8e4 (e4m3) with MatmulPerfMode.DoubleRow: operands
packed [128, 2, free] so each matmul contracts 256 rows.  Measured on
HW: the matmul streams at 0.5 cyc/out-row but every matmul pays a
serial ~134 ns LDWEIGHTS (256-col stationary), which sets the PE floor.
The error budget (rel 2e-2 of an output dominated by ||e||^2+||c||^2
~ 1100) is ~28 absolute; fp8 quantization costs ~12, fp16 output ~1.

Engine split per 2048-centroid chunk (8 chunks):
  PE:  24 DoubleRow matmuls (cross) + 4 ones-matmuls (c_sq reduce)
  ACT: psum evac w/ -e_sq bias -> fp16 for half the i-blocks, negated
       c_sq evacs, and the ct/ct2 load issues (ACT HWDGE ring)
  DVE: fused (psum - e_sq) - c_sq scalar_tensor_tensor for the other
       i-blocks, bf16 squares/adds for c_sq (4x mode), fp16 adds
  GpSimd: nothing — measured ~6x slower than DVE on bulk elementwise
  SP:  output stores + input loads (SP HWDGE ring)

The bf16 ct copy exists because DVE's fp8-input path measured ~5
cycles/element while the all-16-bit path hits the 4x mode; it also
makes c_sq more accurate (no fp8^2 error).  e_sq comes from ACT Square
directly off the phase-1 psum; its ONES-matmul partition-reduce is
emitted after the whole phase-1 loop so the in-order PE queue never
waits on ACT.
"""

import numpy as np

_B, _DIN, _D, _C = 8192, 1024, 768, 16384
_NCORES = 8
_B_LOC = _B // _NCORES

_P = 128  # SBUF/PSUM partitions
_NT = 512  # matmul free-dim tile (one f32 PSUM bank)
_NW = 2048  # main-loop working width (c-chunk)
_NP = 1024  # psum tile width (two banks)

# i-blocks whose psum evac+fixup runs fused on DVE (rest go ACT+add);
# measured: ACT evac ~1.0 ns/col, DVE STT ~1.15, DVE 16-bit ~0.5, so ACT
# takes most of the evacs.  GpSimd gets NOTHING: twice measured (runs 2
# and 5b) its bulk ops run ~5.4 us each and its semaphore handling is
# ~1.2 us, so anything routed there stalls the whole pipeline.
_DVE_I = (3, 6)


def emit_centroid_kernel(tc, xt, w, ct, ct2, out, b_loc, din, d, c):
    """Emit the per-core Tile kernel.

    xt:  [(jx,p), 2, b_loc] fp8e4    (x shard, DoubleRow-packed)
    w:   [(jx,p), 2, d] fp8e4        (64*W, DoubleRow-packed)
    ct:  [(jd,p), 2, c] fp8e4        (centroids.T, DoubleRow-packed)
    ct2: [(jd,p), 2, c] bf16         (same layout, feeds the c_sq squares)
    out: [b_loc, c] fp16
    """
    from concourse import mybir
    from concourse.alu_op_type import AluOpType
    from concourse.masks import make_identity

    nc = tc.nc
    e4 = mybir.dt.float8e4
    bf16 = mybir.dt.bfloat16
    f16 = mybir.dt.float16
    f32 = mybir.dt.float32
    AF = mybir.ActivationFunctionType
    DR = mybir.MatmulPerfMode.DoubleRow

    jx = din // (2 * _P)  # k-pairs over d_in
    jd = d // (2 * _P)  # k-pairs over d (embedding)
    md = d // _P  # 128-blocks over d
    mb = b_loc // _P  # tiles over local batch
    npair = c // _NW  # c-chunks

    with (
        tc.tile_pool(name="persist", bufs=1) as persist,
        tc.tile_pool(name="ct_in", bufs=9) as ct_pool,
        tc.tile_pool(name="ct2_in", bufs=6) as ct2_pool,
        tc.tile_pool(name="sq", bufs=3) as sq_pool,
        tc.tile_pool(name="csum", bufs=2) as csum_pool,
        tc.tile_pool(name="csqs", bufs=3) as csq_pool,
        tc.tile_pool(name="t1", bufs=3) as t1_pool,
        tc.tile_pool(name="outs", bufs=5) as out_pool,
        tc.tile_pool(name="scratch", bufs=2) as scratch,
    ):
        # ---- persistent SBUF tensors ----
        xt_s = [persist.tile([_P, 2, b_loc], e4, name=f"xt{j}", tag=f"xt{j}") for j in range(jx)]
        w_s = [persist.tile([_P, 2, d], e4, name=f"w{j}", tag=f"w{j}") for j in range(jx)]
        et2_s = [persist.tile([_P, 2, b_loc], e4, name=f"et{j}", tag=f"et{j}") for j in range(jd)]
        negesq = persist.tile([_P, mb], f32, name="negesq", tag="negesq")
        ones = persist.tile([_P, _P], bf16, name="ones", tag="ones")
        ident = persist.tile([_P, _P], f32, name="ident", tag="ident")

        for j in range(jx):
            nc.sync.dma_start(w_s[j][:], w[j * _P : (j + 1) * _P, :, :])
            nc.sync.dma_start(xt_s[j][:], xt[j * _P : (j + 1) * _P, :, :])
        # emitted after the input loads: only needed by e_sq/c_sq, and
        # emitting them first would delay the PE-critical xt/w arrival
        nc.vector.memset(ones[:], 1.0)
        make_identity(nc, ident[:])

        def load_ct(n):
            # both centroid copies ride the ACT HWDGE ring, leaving the
            # SP ring to the (larger) output stores
            csl = slice(n * _NW, (n + 1) * _NW)
            tiles = []
            for j in range(jd):
                t = ct_pool.tile([_P, 2, _NW], e4, name=f"ct{j}", tag="ct")
                nc.scalar.dma_start(t[:], ct[j * _P : (j + 1) * _P, :, csl])
                tiles.append(t)
            tiles2 = []
            for j in range(jd):
                t2 = ct2_pool.tile([_P, 2, _NW], bf16, name=f"cu{j}", tag="cu")
                nc.scalar.dma_start(t2[:], ct2[j * _P : (j + 1) * _P, :, csl])
                tiles2.append(t2)
            return tiles, tiles2

        # the first two chunks' centroid loads are issued BEFORE the
        # prologue is emitted: the Scalar engine queue is in-order, so
        # issuing them here (while it is still empty) lets the DMAs run
        # under the prologue instead of queueing behind its psum evacs.
        ct_q = [load_ct(0), load_ct(1)]

        # ---- prologue (own PSUM scope, freed before the main loop) ----
        with tc.tile_pool(name="ps_pro", bufs=2, space="PSUM") as ps_pro:
            # phase 1: et2 = fp8(2 * (W.T @ xT));  psum = 64 * eT.
            # sqe = (2e)^2 via ACT Square off the same psum.
            sqes = []
            for m in range(md):
                pt = ps_pro.tile([_P, b_loc], f32, name="pro", tag="pro")
                for nb in range(b_loc // _NT):
                    bs = slice(nb * _NT, (nb + 1) * _NT)
                    for j in range(jx):
                        nc.tensor.matmul(
                            pt[:, bs],
                            w_s[j][:, :, m * _P : (m + 1) * _P],
                            xt_s[j][:, :, bs],
                            start=(j == 0),
                            stop=(j == jx - 1),
                            perf_mode=DR,
                        )
                nc.scalar.activation(
                    et2_s[m // 2][:, m % 2, :], pt[:], AF.Copy, scale=0.03125
                )
                sqe = scratch.tile([_P, b_loc], bf16, name="sqe", tag="sqe", bufs=md)
                nc.scalar.activation(sqe[:], pt[:], AF.Square, scale=0.03125)
                sqes.append(sqe)
            # phase 2 (emitted after the whole m-loop so the PE never
            # head-of-line waits on the ACT Squares): ONES-matmul
            # partition-reduce of sum_m sqe -> 4*e_sq replicated.
            pesq = ps_pro.tile([_P, b_loc], f32, name="pesq", tag="pesq", bufs=1)
            for m in range(md):
                for nb in range(b_loc // _NT):
                    bs = slice(nb * _NT, (nb + 1) * _NT)
                    nc.tensor.matmul(
                        pesq[:, bs],
                        ones[:],
                        sqes[m][:, bs],
                        start=(m == 0),
                        stop=(m == md - 1),
                    )
            # PE-transpose each [128, 128] slice so e_sq lands
            # per-partition; sqe held (2e)^2 so the scale is -1/4.
            esq_rep = scratch.tile([_P, b_loc], f32, name="esq_rep", tag="esq_rep")
            nc.scalar.activation(esq_rep[:], pesq[:], AF.Copy)
            for i in range(mb):
                ptr = ps_pro.tile([_P, _P], f32, name="ptr", tag="ptr")
                nc.tensor.transpose(ptr[:], esq_rep[:, i * _P : (i + 1) * _P], ident[:])
                nc.scalar.activation(negesq[:, i : i + 1], ptr[:, 0:1], AF.Copy, scale=-0.25)

        # ---- main loop over c-chunks (2048 centroids each) ----
        with (
            tc.tile_pool(name="ps_big", bufs=3, space="PSUM") as ps_big,
            tc.tile_pool(name="ps_csq", bufs=1, space="PSUM") as ps_csq,
        ):
            def csq_front(ct2_t):
                # sum_k ct^2: all-bf16 DVE squares + adds
                sqs = []
                for j in range(jd):
                    sq_t = sq_pool.tile([_P, 2, _NW], bf16, name="sqc", tag="sqc")
                    nc.vector.tensor_mul(sq_t[:], ct2_t[j][:], ct2_t[j][:])
                    sqs.append(sq_t)
                nc.vector.tensor_add(sqs[0][:], sqs[0][:], sqs[1][:])
                nc.vector.tensor_add(sqs[0][:], sqs[0][:], sqs[2][:])
                csum = csum_pool.tile([_P, _NW], bf16, name="csum", tag="csum")
                nc.vector.tensor_add(csum[:], sqs[0][:, 0, :], sqs[0][:, 1, :])
                return csum

            def csq_back(csum):
                # ONES-matmul partition-reduce, ACT-evacuated NEGATED to
                # fp16 so the output fixup is an add. The 2-bank psum
                # tile is reused for both 1024-halves.
                csq_s = csq_pool.tile([_P, _NW], f16, name="csq_s", tag="csq_s")
                for g in range(_NW // _NP):
                    pcs = ps_csq.tile([_P, _NP], f32, name="csq", tag="csq")
                    for h in range(2):
                        nc.tensor.matmul(
                            pcs[:, h * _NT : (h + 1) * _NT],
                            ones[:],
                            csum[:, g * _NP + h * _NT : g * _NP + (h + 1) * _NT],
                            start=True,
                            stop=True,
                        )
                    nc.scalar.activation(
                        csq_s[:, g * _NP : (g + 1) * _NP], pcs[:], AF.Copy, scale=-1.0
                    )
                return csq_s

            # csq's DVE work starts as soon as the chunk's ct lands, but its
            # PE matmuls + ACT evac are emitted LATE (at i == mb-2) so they
            # never head-of-line block the in-order PE queue while the DVE
            # chain finishes.  ct is prefetched TWO chunks ahead so loads
            # never arrive late even when the Scalar queue is backed up.
            csq_cur = csq_back(csq_front(ct_q[0][1]))
            for n in range(npair):
                csl = slice(n * _NW, (n + 1) * _NW)
                ct_cur, ct2_cur = ct_q[n % 2]
                if n + 2 < npair:
                    ct_q[n % 2] = load_ct(n + 2)
                csum_nxt = csq_front(ct_q[(n + 1) % 2][1]) if n + 1 < npair else None

                for i in range(mb):
                    # j-outer / seg-inner: the 4 psum segments share the
                    # stationary operand of each j
                    pbs = [
                        ps_big.tile([_P, _NP], f32, name="bigA", tag="big"),
                        ps_big.tile([_P, _NP], f32, name="bigB", tag="big"),
                    ]
                    for j in range(jd):
                        lhsT = et2_s[j][:, :, i * _P : (i + 1) * _P]
                        for s in range(_NW // _NT):
                            nc.tensor.matmul(
                                pbs[s // 2][:, (s % 2) * _NT : (s % 2 + 1) * _NT],
                                lhsT,
                                ct_cur[j][:, :, s * _NT : (s + 1) * _NT],
                                start=(j == 0),
                                stop=(j == jd - 1),
                                perf_mode=DR,
                            )
                    ot = out_pool.tile([_P, _NW], f16, name="ot", tag="ot")
                    if i in _DVE_I:
                        # fused psum evac + e_sq bias + c_sq add on DVE
                        for g in range(2):
                            nc.vector.scalar_tensor_tensor(
                                ot[:, g * _NP : (g + 1) * _NP],
                                pbs[g][:],
                                negesq[:, i : i + 1],
                                csq_cur[:, g * _NP : (g + 1) * _NP],
                                AluOpType.add,
                                AluOpType.add,
                            )
                    else:
                        t1 = t1_pool.tile([_P, _NW], f16, name="t1", tag="t1")
                        for g in range(2):
                            nc.scalar.activation(
                                t1[:, g * _NP : (g + 1) * _NP],
                                pbs[g][:],
                                AF.Identity,
                                bias=negesq[:, i : i + 1],
                            )
                        nc.vector.tensor_add(ot[:], t1[:], csq_cur[:])
                    # the last chunk splits its stores across both HWDGE
                    # rings — the loads are done, and it halves the drain
                    # tail after the final compute
                    if n == npair - 1 and i % 2 == 1:
                        nc.scalar.dma_start(out[i * _P : (i + 1) * _P, csl], ot[:])
                    else:
                        nc.sync.dma_start(out[i * _P : (i + 1) * _P, csl], ot[:])
                    if i == mb - 2 and csum_nxt is not None:
                        csq_nxt = csq_back(csum_nxt)
                if csum_nxt is not None:
                    csq_cur = csq_nxt


def build_nc(b_loc=_B_LOC, din=_DIN, d=_D, c=_C):
    import concourse.tile as tile
    from concourse import bacc, mybir

    nc = bacc.Bacc("TRN2", target_bir_lowering=False, debug=False)
    jx = din // (2 * _P)
    jd = d // (2 * _P)
    xt = nc.declare_dram_parameter("xt", [jx * _P, 2, b_loc], mybir.dt.float8e4, isOutput=False)
    w = nc.declare_dram_parameter("w", [jx * _P, 2, d], mybir.dt.float8e4, isOutput=False)
    ct = nc.declare_dram_parameter("ct", [jd * _P, 2, c], mybir.dt.float8e4, isOutput=False)
    ct2 = nc.declare_dram_parameter("ct2", [jd * _P, 2, c], mybir.dt.bfloat16, isOutput=False)
    out = nc.declare_dram_parameter("out", [b_loc, c], mybir.dt.float16, isOutput=True)
    with tile.TileContext(nc) as tc:
        emit_centroid_kernel(tc, xt.ap(), w.ap(), ct.ap(), ct2.ap(), out.ap(), b_loc, din, d, c)
    nc.compile()
    return nc


def _pack_pairs(a2d, dtype):
    """[K, F] -> [(j,p), 2, F] DoubleRow pair layout, row (2j+k2)*128+p."""
    k, f = a2d.shape
    j = k // (2 * _P)
    return np.ascontiguousarray(
        a2d.reshape(j, 2, _P, f).transpose(0, 2, 1, 3).reshape(j * _P, 2, f)
    ).astype(dtype)


def make_in_maps(x, W, centroids, b_loc=_B_LOC, n_cores=_NCORES):
    import ml_dtypes

    e4 = ml_dtypes.float8_e4m3

    x = np.asarray(x, dtype=np.float32)
    W = np.asarray(W, dtype=np.float32)
    centroids = np.asarray(centroids, dtype=np.float32)

    # W is pre-scaled by 64 so its ~0.02-sigma values land in fp8e4's
    # normal range (min normal 2^-6); the kernel rescales by 2/64.
    w_p = _pack_pairs(W * 64.0, e4)  # [(jx,p), 2, D]
    ctt = np.ascontiguousarray(centroids.T)
    ct_p = _pack_pairs(ctt, e4)  # [(jd,p), 2, C]
    ct2_p = _pack_pairs(ctt, ml_dtypes.bfloat16)
    xt_full = np.ascontiguousarray(x.T)  # [DIN, B]

    maps = []
    for i in range(n_cores):
        xt_p = _pack_pairs(xt_full[:, i * b_loc : (i + 1) * b_loc], e4)
        maps.append({"xt": xt_p, "w": w_p, "ct": ct_p, "ct2": ct2_p})
    return maps


_NC_CACHE = {}


def kernel(x, W, centroids):
    from concourse.bass_utils import run_bass_kernel_spmd

    if "nc" not in _NC_CACHE:
        _NC_CACHE["nc"] = build_nc()
    nc = _NC_CACHE["nc"]

    in_maps = make_in_maps(x, W, centroids)
    res = run_bass_kernel_spmd(nc, in_maps, list(range(_NCORES)))
    return np.concatenate(
        [res.results[i]["out"].astype(np.float32) for i in range(_NCORES)], axis=0
    )


# revision 38
# speedup vs baseline: 1.0906x; 1.0083x over previous
"""Trainium2 Bass kernel for nn_CentroidModel (retrieval_knn).

Computes out = -(||e||^2 + ||c||^2 - 2 e.c) with e = x @ W, over 8
NeuronCores, data-parallel on the batch dim (x rows sharded; W and
centroids replicated).

The big GEMM runs in fp8e4 (e4m3) with MatmulPerfMode.DoubleRow:
operands packed [128, 2, free] so each matmul contracts 256 rows.
Measured on HW: the matmul streams at 0.5 cyc/out-row but every matmul
pays a serial ~134 ns LDWEIGHTS (256-col stationary), which sets the
PE floor (~210 us/core).  The error budget (rel 2e-2 of an output
dominated by ||e||^2+||c||^2 ~ 1100) is ~28 absolute; fp8 quantization
of e and c costs ~8, fp16 output rounding ~1.

Engine split per 2048-centroid chunk (8 chunks):
  PE:  24 DoubleRow matmuls (cross) + 4 ones-matmuls (c_sq reduce)
  ACT: 16 psum evacs w/ -e_sq bias -> fp16, 2 negated c_sq evacs,
       ct2 load issues (ACT HWDGE ring)
  DVE: 3 squares + 3 adds (bf16) for c_sq + 8 output adds (fp16)
  GpSimd: nothing — twice measured ~6x slower than DVE with ~1.2 us
       semaphore handling; anything routed there stalls the pipeline
  SP:  output stores + input + ct fp8 loads (SP HWDGE ring)

The bf16 ct copy exists because DVE's fp8-input path measured ~5
cycles/element while the all-16-bit path is ~4-8x faster; it also
makes c_sq more accurate (no fp8^2 error).  e_sq comes from ACT Square
directly off the phase-1 psum (fp16 phase-1: same PE rate as bf16,
10-bit mantissa).
"""

import numpy as np

_B, _DIN, _D, _C = 8192, 1024, 768, 16384
_NCORES = 8
_B_LOC = _B // _NCORES

_P = 128  # SBUF/PSUM partitions
_NT = 512  # matmul free-dim tile (one f32 PSUM bank)
_NW = 2048  # main-loop working width (c-chunk)
_NP = 1024  # psum tile width (two banks)

_JD = _D // (2 * _P)  # k-pairs over d (3)


def emit_centroid_kernel(tc, xt, w, ct, ct2, out, b_loc, din, d, c):
    """Emit the per-core Tile kernel.

    xt:  [din, b_loc] fp16           (x shard, pre-transposed)
    w:   [din, d] fp16
    ct:  [(jd,p), 2, c] fp8e4        (centroids.T, DoubleRow-packed)
    ct2: [(jd,p), 2, c] bf16         (same layout, feeds the c_sq squares)
    out: [b_loc, c] fp16
    """
    from concourse import mybir
    from concourse.masks import make_identity

    nc = tc.nc
    e4 = mybir.dt.float8e4
    bf16 = mybir.dt.bfloat16
    f16 = mybir.dt.float16
    f32 = mybir.dt.float32
    AF = mybir.ActivationFunctionType
    DR = mybir.MatmulPerfMode.DoubleRow

    kd = din // _P  # k-tiles over d_in
    jd = d // (2 * _P)  # k-pairs over d (embedding)
    md = d // _P  # 128-blocks over d
    mb = b_loc // _P  # tiles over local batch
    npair = c // _NW  # c-chunks

    with (
        tc.tile_pool(name="persist", bufs=1) as persist,
        tc.tile_pool(name="ct_in", bufs=6) as ct_pool,
        tc.tile_pool(name="ct2_in", bufs=4) as ct2_pool,
        tc.tile_pool(name="sq", bufs=3) as sq_pool,
        tc.tile_pool(name="csum", bufs=2) as csum_pool,
        tc.tile_pool(name="csqs", bufs=3) as csq_pool,
        tc.tile_pool(name="t1", bufs=4) as t1_pool,
        tc.tile_pool(name="outs", bufs=7) as out_pool,
        tc.tile_pool(name="scratch", bufs=2) as scratch,
    ):
        # ---- persistent SBUF tensors ----
        xt_s = [persist.tile([_P, b_loc], f16, name=f"xt{k}", tag=f"xt{k}") for k in range(kd)]
        w_s = [persist.tile([_P, d], f16, name=f"w{k}", tag=f"w{k}") for k in range(kd)]
        et2_s = [persist.tile([_P, 2, b_loc], e4, name=f"et{j}", tag=f"et{j}") for j in range(jd)]
        negesq = persist.tile([_P, mb], f32, name="negesq", tag="negesq")
        ones = persist.tile([_P, _P], bf16, name="ones", tag="ones")
        ident = persist.tile([_P, _P], f32, name="ident", tag="ident")

        for k in range(kd):
            nc.sync.dma_start(xt_s[k][:], xt[k * _P : (k + 1) * _P, :])
            nc.sync.dma_start(w_s[k][:], w[k * _P : (k + 1) * _P, :])
        # emitted after the input loads: only needed by e_sq/c_sq, and
        # emitting them first would delay the PE-critical xt/w arrival
        nc.vector.memset(ones[:], 1.0)
        make_identity(nc, ident[:])

        # ---- prologue (own PSUM scope, freed before the main loop) ----
        with tc.tile_pool(name="ps_pro", bufs=2, space="PSUM") as ps_pro:
            # phase 1: et2 = fp8(2 * (W.T @ xT)) in fp16; sqe = (2e)^2 via
            # ACT Square off the same psum.
            sqes = []
            for m in range(md):
                pt = ps_pro.tile([_P, b_loc], f32, name="pro", tag="pro")
                for nb in range(b_loc // _NT):
                    bs = slice(nb * _NT, (nb + 1) * _NT)
                    for k in range(kd):
                        nc.tensor.matmul(
                            pt[:, bs],
                            w_s[k][:, m * _P : (m + 1) * _P],
                            xt_s[k][:, bs],
                            start=(k == 0),
                            stop=(k == kd - 1),
                        )
                nc.scalar.activation(
                    et2_s[m // 2][:, m % 2, :], pt[:], AF.Copy, scale=2.0
                )
                sqe = scratch.tile([_P, b_loc], bf16, name="sqe", tag="sqe", bufs=md)
                nc.scalar.activation(sqe[:], pt[:], AF.Square, scale=2.0)
                sqes.append(sqe)
            # phase 2 (emitted after the whole m-loop so the PE never
            # head-of-line waits on the ACT Squares): ONES-matmul
            # partition-reduce of sum_m sqe -> 4*e_sq replicated.
            pesq = ps_pro.tile([_P, b_loc], f32, name="pesq", tag="pesq", bufs=1)
            for m in range(md):
                for nb in range(b_loc // _NT):
                    bs = slice(nb * _NT, (nb + 1) * _NT)
                    nc.tensor.matmul(
                        pesq[:, bs],
                        ones[:],
                        sqes[m][:, bs],
                        start=(m == 0),
                        stop=(m == md - 1),
                    )
            # PE-transpose each [128, 128] slice so e_sq lands
            # per-partition, scale by -1/4.
            esq_rep = scratch.tile([_P, b_loc], f32, name="esq_rep", tag="esq_rep")
            nc.scalar.activation(esq_rep[:], pesq[:], AF.Copy)
            for i in range(mb):
                ptr = ps_pro.tile([_P, _P], f32, name="ptr", tag="ptr")
                nc.tensor.transpose(ptr[:], esq_rep[:, i * _P : (i + 1) * _P], ident[:])
                nc.scalar.activation(negesq[:, i : i + 1], ptr[:, 0:1], AF.Copy, scale=-0.25)

        # ---- main loop over c-chunks (2048 centroids each) ----
        with (
            tc.tile_pool(name="ps_big", bufs=3, space="PSUM") as ps_big,
            tc.tile_pool(name="ps_csq", bufs=1, space="PSUM") as ps_csq,
        ):
            def load_ct(n):
                # fp8 copy on the SP ring; bf16 squaring copy on the ACT ring
                csl = slice(n * _NW, (n + 1) * _NW)
                tiles = []
                for j in range(jd):
                    t = ct_pool.tile([_P, 2, _NW], e4, name=f"ct{j}", tag="ct")
                    nc.sync.dma_start(t[:], ct[j * _P : (j + 1) * _P, :, csl])
                    tiles.append(t)
                tiles2 = []
                for j in range(jd):
                    t2 = ct2_pool.tile([_P, 2, _NW], bf16, name=f"cu{j}", tag="cu")
                    nc.scalar.dma_start(t2[:], ct2[j * _P : (j + 1) * _P, :, csl])
                    tiles2.append(t2)
                return tiles, tiles2

            def csq_front(ct2_t):
                # sum_k ct^2: all-bf16 DVE squares + adds
                sqs = []
                for j in range(jd):
                    sq_t = sq_pool.tile([_P, 2, _NW], bf16, name="sqc", tag="sqc")
                    nc.vector.tensor_mul(sq_t[:], ct2_t[j][:], ct2_t[j][:])
                    sqs.append(sq_t)
                nc.vector.tensor_add(sqs[0][:], sqs[0][:], sqs[1][:])
                nc.vector.tensor_add(sqs[0][:], sqs[0][:], sqs[2][:])
                csum = csum_pool.tile([_P, _NW], bf16, name="csum", tag="csum")
                nc.vector.tensor_add(csum[:], sqs[0][:, 0, :], sqs[0][:, 1, :])
                return csum

            def csq_back(csum):
                # ONES-matmul partition-reduce, ACT-evacuated NEGATED to
                # fp16 so the output fixup is a single DVE add. The
                # 2-bank psum tile is reused for both 1024-halves.
                csq_s = csq_pool.tile([_P, _NW], f16, name="csq_s", tag="csq_s")
                for g in range(_NW // _NP):
                    pcs = ps_csq.tile([_P, _NP], f32, name="csq", tag="csq")
                    for h in range(2):
                        nc.tensor.matmul(
                            pcs[:, h * _NT : (h + 1) * _NT],
                            ones[:],
                            csum[:, g * _NP + h * _NT : g * _NP + (h + 1) * _NT],
                            start=True,
                            stop=True,
                        )
                    nc.scalar.activation(
                        csq_s[:, g * _NP : (g + 1) * _NP], pcs[:], AF.Copy, scale=-1.0
                    )
                return csq_s

            # csq's DVE work starts as soon as the chunk's ct lands, but its
            # PE matmuls + ACT evac are emitted LATE (at i == mb-2) so they
            # never head-of-line block the in-order PE queue while the DVE
            # chain finishes.  ct loads are issued before the chunk's stores
            # so they stay ahead on the shared SP HWDGE ring.
            ct_cur, ct2_cur = load_ct(0)
            csq_cur = csq_back(csq_front(ct2_cur))
            for n in range(npair):
                csl = slice(n * _NW, (n + 1) * _NW)
                ct_nxt, ct2_nxt = load_ct(n + 1) if n + 1 < npair else (None, None)
                csum_nxt = csq_front(ct2_nxt) if ct_nxt is not None else None

                for i in range(mb):
                    # j-outer / seg-inner: the 4 psum segments share the
                    # stationary operand of each j
                    pbs = [
                        ps_big.tile([_P, _NP], f32, name="bigA", tag="big"),
                        ps_big.tile([_P, _NP], f32, name="bigB", tag="big"),
                    ]
                    for j in range(jd):
                        lhsT = et2_s[j][:, :, i * _P : (i + 1) * _P]
                        for s in range(_NW // _NT):
                            nc.tensor.matmul(
                                pbs[s // 2][:, (s % 2) * _NT : (s % 2 + 1) * _NT],
                                lhsT,
                                ct_cur[j][:, :, s * _NT : (s + 1) * _NT],
                                start=(j == 0),
                                stop=(j == jd - 1),
                                perf_mode=DR,
                            )
                    t1 = t1_pool.tile([_P, _NW], f16, name="t1", tag="t1")
                    for g in range(2):
                        nc.scalar.activation(
                            t1[:, g * _NP : (g + 1) * _NP],
                            pbs[g][:],
                            AF.Identity,
                            bias=negesq[:, i : i + 1],
                        )
                    ot = out_pool.tile([_P, _NW], f16, name="ot", tag="ot")
                    nc.vector.tensor_add(ot[:], t1[:], csq_cur[:])
                    nc.sync.dma_start(out[i * _P : (i + 1) * _P, csl], ot[:])
                    if i == mb - 2 and csum_nxt is not None:
                        csq_nxt = csq_back(csum_nxt)
                if ct_nxt is not None:
                    ct_cur, ct2_cur, csq_cur = ct_nxt, ct2_nxt, csq_nxt


def build_nc(b_loc=_B_LOC, din=_DIN, d=_D, c=_C):
    import concourse.tile as tile
    from concourse import bacc, mybir

    nc = bacc.Bacc("TRN2", target_bir_lowering=False, debug=False)
    jd = d // (2 * _P)
    xt = nc.declare_dram_parameter("xt", [din, b_loc], mybir.dt.float16, isOutput=False)
    w = nc.declare_dram_parameter("w", [din, d], mybir.dt.float16, isOutput=False)
    ct = nc.declare_dram_parameter("ct", [jd * _P, 2, c], mybir.dt.float8e4, isOutput=False)
    ct2 = nc.declare_dram_parameter("ct2", [jd * _P, 2, c], mybir.dt.bfloat16, isOutput=False)
    out = nc.declare_dram_parameter("out", [b_loc, c], mybir.dt.float16, isOutput=True)
    with tile.TileContext(nc) as tc:
        emit_centroid_kernel(tc, xt.ap(), w.ap(), ct.ap(), ct2.ap(), out.ap(), b_loc, din, d, c)
    nc.compile()
    return nc


def _pack_pairs(a2d, dtype):
    """[K, F] -> [(j,p), 2, F] DoubleRow pair layout, row (2j+k2)*128+p."""
    k, f = a2d.shape
    j = k // (2 * _P)
    return np.ascontiguousarray(
        a2d.reshape(j, 2, _P, f).transpose(0, 2, 1, 3).reshape(j * _P, 2, f)
    ).astype(dtype)


def make_in_maps(x, W, centroids, b_loc=_B_LOC, n_cores=_NCORES):
    import ml_dtypes

    e4 = ml_dtypes.float8_e4m3

    x = np.asarray(x, dtype=np.float32)
    W = np.asarray(W, dtype=np.float32)
    centroids = np.asarray(centroids, dtype=np.float32)

    w_f16 = W.astype(np.float16)  # [DIN, D]
    ctt = np.ascontiguousarray(centroids.T)
    ct_p = _pack_pairs(ctt, e4)  # [(jd,p), 2, C]
    ct2_p = _pack_pairs(ctt, ml_dtypes.bfloat16)
    xt_full = np.ascontiguousarray(x.T).astype(np.float16)  # [DIN, B]

    maps = []
    for i in range(n_cores):
        xt_p = np.ascontiguousarray(xt_full[:, i * b_loc : (i + 1) * b_loc])
        maps.append({"xt": xt_p, "w": w_f16, "ct": ct_p, "ct2": ct2_p})
    return maps


_NC_CACHE = {}


def kernel(x, W, centroids):
    from concourse.bass_utils import run_bass_kernel_spmd

    if "nc" not in _NC_CACHE:
        _NC_CACHE["nc"] = build_nc()
    nc = _NC_CACHE["nc"]

    in_maps = make_in_maps(x, W, centroids)
    res = run_bass_kernel_spmd(nc, in_maps, list(range(_NCORES)))
    return np.concatenate(
        [res.results[i]["out"].astype(np.float32) for i in range(_NCORES)], axis=0
    )


# revision 39
# speedup vs baseline: 1.3399x; 1.2285x over previous
"""Trainium2 Bass kernel for nn_CentroidModel (retrieval_knn).

HW-validated fallback (run 1): 281.8 us, rel err 1.256e-2.

Computes out = -(||e||^2 + ||c||^2 - 2 e.c) with e = x @ W, over 8
NeuronCores, data-parallel on the batch dim (x rows sharded; W and
centroids replicated).  Big GEMM in fp8e4 DoubleRow; fp16 phase-1;
fp16 output.
"""

import numpy as np

_B, _DIN, _D, _C = 8192, 1024, 768, 16384
_NCORES = 8
_B_LOC = _B // _NCORES

_P = 128
_NT = 512
_NW = 1024


def emit_centroid_kernel(tc, xt, w, ct, out, b_loc, din, d, c):
    from concourse import mybir
    from concourse.masks import make_identity

    nc = tc.nc
    e4 = mybir.dt.float8e4
    bf16 = mybir.dt.bfloat16
    f16 = mybir.dt.float16
    f32 = mybir.dt.float32
    AF = mybir.ActivationFunctionType
    DR = mybir.MatmulPerfMode.DoubleRow

    kd = din // _P
    jd = d // (2 * _P)
    md = d // _P
    mb = b_loc // _P
    npair = c // _NW

    with (
        tc.tile_pool(name="persist", bufs=1) as persist,
        tc.tile_pool(name="ct_in", bufs=9) as ct_pool,
        tc.tile_pool(name="sq", bufs=8) as sq_pool,
        tc.tile_pool(name="csum", bufs=2) as csum_pool,
        tc.tile_pool(name="csqs", bufs=3) as csq_pool,
        tc.tile_pool(name="t1", bufs=6) as t1_pool,
        tc.tile_pool(name="outs", bufs=10) as out_pool,
        tc.tile_pool(name="scratch", bufs=2) as scratch,
    ):
        xt_s = [persist.tile([_P, b_loc], f16, name=f"xt{k}", tag=f"xt{k}") for k in range(kd)]
        w_s = [persist.tile([_P, d], f16, name=f"w{k}", tag=f"w{k}") for k in range(kd)]
        et2_s = [persist.tile([_P, 2, b_loc], e4, name=f"et{j}", tag=f"et{j}") for j in range(jd)]
        negesq = persist.tile([_P, mb], f32, name="negesq", tag="negesq")
        ones = persist.tile([_P, _P], bf16, name="ones", tag="ones")
        ident = persist.tile([_P, _P], f32, name="ident", tag="ident")

        for k in range(kd):
            nc.sync.dma_start(xt_s[k][:], xt[k * _P : (k + 1) * _P, :])
            nc.sync.dma_start(w_s[k][:], w[k * _P : (k + 1) * _P, :])
        nc.vector.memset(ones[:], 1.0)
        make_identity(nc, ident[:])

        with tc.tile_pool(name="ps_pro", bufs=2, space="PSUM") as ps_pro:
            for m in range(md):
                pt = ps_pro.tile([_P, b_loc], f32, name="pro", tag="pro")
                for nb in range(b_loc // _NT):
                    bs = slice(nb * _NT, (nb + 1) * _NT)
                    for k in range(kd):
                        nc.tensor.matmul(
                            pt[:, bs],
                            w_s[k][:, m * _P : (m + 1) * _P],
                            xt_s[k][:, bs],
                            start=(k == 0),
                            stop=(k == kd - 1),
                        )
                nc.scalar.activation(
                    et2_s[m // 2][:, m % 2, :], pt[:], AF.Copy, scale=2.0
                )

            pesq = ps_pro.tile([_P, b_loc], f32, name="pesq", tag="pesq", bufs=1)
            for j in range(jd):
                sqe = scratch.tile([_P, 2, b_loc], bf16, name="sqe", tag="sqe")
                nc.vector.tensor_mul(sqe[:], et2_s[j][:], et2_s[j][:])
                for k2 in range(2):
                    for nb in range(b_loc // _NT):
                        bs = slice(nb * _NT, (nb + 1) * _NT)
                        nc.tensor.matmul(
                            pesq[:, bs],
                            ones[:],
                            sqe[:, k2, bs],
                            start=(j == 0 and k2 == 0),
                            stop=(j == jd - 1 and k2 == 1),
                        )
            esq_rep = scratch.tile([_P, b_loc], f32, name="esq_rep", tag="esq_rep")
            nc.scalar.activation(esq_rep[:], pesq[:], AF.Copy)
            for i in range(mb):
                ptr = ps_pro.tile([_P, _P], f32, name="ptr", tag="ptr")
                nc.tensor.transpose(ptr[:], esq_rep[:, i * _P : (i + 1) * _P], ident[:])
                nc.scalar.activation(negesq[:, i : i + 1], ptr[:, 0:1], AF.Copy, scale=-0.25)

        with (
            tc.tile_pool(name="ps_big", bufs=3, space="PSUM") as ps_big,
            tc.tile_pool(name="ps_csq", bufs=1, space="PSUM") as ps_csq,
        ):
            def load_ct(n):
                csl = slice(n * _NW, (n + 1) * _NW)
                tiles = []
                for j in range(jd):
                    t = ct_pool.tile([_P, 2, _NW], e4, name=f"ct{j}", tag="ct")
                    nc.sync.dma_start(t[:], ct[j * _P : (j + 1) * _P, :, csl])
                    tiles.append(t)
                return tiles

            def emit_csq(ct_t):
                sqs = []
                for j in range(jd):
                    sq_t = sq_pool.tile([_P, 2, _NW], bf16, name="sqc", tag="sqc")
                    nc.vector.tensor_mul(sq_t[:], ct_t[j][:], ct_t[j][:])
                    sqs.append(sq_t)
                nc.vector.tensor_add(sqs[0][:], sqs[0][:], sqs[1][:])
                nc.vector.tensor_add(sqs[0][:], sqs[0][:], sqs[2][:])
                csum = csum_pool.tile([_P, _NW], bf16, name="csum", tag="csum")
                nc.vector.tensor_add(csum[:], sqs[0][:, 0, :], sqs[0][:, 1, :])
                pcs = ps_csq.tile([_P, _NW], f32, name="csq", tag="csq")
                csq_s = csq_pool.tile([_P, _NW], f16, name="csq_s", tag="csq_s")
                for h in range(2):
                    hs = slice(h * _NT, (h + 1) * _NT)
                    nc.tensor.matmul(pcs[:, hs], ones[:], csum[:, hs], start=True, stop=True)
                nc.scalar.activation(csq_s[:], pcs[:], AF.Copy)
                return csq_s

            pending_stores = []
            ct_cur = load_ct(0)
            csq_cur = emit_csq(ct_cur)
            for n in range(npair):
                csl = slice(n * _NW, (n + 1) * _NW)
                ct_nxt = load_ct(n + 1) if n + 1 < npair else None
                for dst, src_t in pending_stores:
                    nc.sync.dma_start(dst, src_t[:])
                pending_stores = []

                for i in range(mb):
                    pb = ps_big.tile([_P, _NW], f32, name="big", tag="big")
                    for j in range(jd):
                        lhsT = et2_s[j][:, :, i * _P : (i + 1) * _P]
                        nc.tensor.matmul(
                            pb[:, 0:_NT], lhsT, ct_cur[j][:, :, 0:_NT],
                            start=(j == 0), stop=(j == jd - 1), perf_mode=DR,
                        )
                        nc.tensor.matmul(
                            pb[:, _NT:_NW], lhsT, ct_cur[j][:, :, _NT:_NW],
                            start=(j == 0), stop=(j == jd - 1), perf_mode=DR,
                        )
                    t1 = t1_pool.tile([_P, _NW], f16, name="t1", tag="t1")
                    nc.scalar.activation(
                        t1[:], pb[:], AF.Identity, bias=negesq[:, i : i + 1]
                    )
                    ot = out_pool.tile([_P, _NW], f16, name="ot", tag="ot")
                    nc.vector.tensor_sub(ot[:], t1[:], csq_cur[:])
                    if n == npair - 1:
                        nc.sync.dma_start(out[i * _P : (i + 1) * _P, csl], ot[:])
                    else:
                        pending_stores.append((out[i * _P : (i + 1) * _P, csl], ot))
                    if i == 0 and ct_nxt is not None:
                        csq_nxt = emit_csq(ct_nxt)
                if ct_nxt is not None:
                    ct_cur, csq_cur = ct_nxt, csq_nxt
            for dst, src_t in pending_stores:
                nc.sync.dma_start(dst, src_t[:])


def build_nc(b_loc=_B_LOC, din=_DIN, d=_D, c=_C):
    import concourse.tile as tile
    from concourse import bacc, mybir

    nc = bacc.Bacc("TRN2", target_bir_lowering=False, debug=False)
    jd = d // (2 * _P)
    xt = nc.declare_dram_parameter("xt", [din, b_loc], mybir.dt.float16, isOutput=False)
    w = nc.declare_dram_parameter("w", [din, d], mybir.dt.float16, isOutput=False)
    ct = nc.declare_dram_parameter("ct", [jd * _P, 2, c], mybir.dt.float8e4, isOutput=False)
    out = nc.declare_dram_parameter("out", [b_loc, c], mybir.dt.float16, isOutput=True)
    with tile.TileContext(nc) as tc:
        emit_centroid_kernel(tc, xt.ap(), w.ap(), ct.ap(), out.ap(), b_loc, din, d, c)
    nc.compile()
    return nc


def _pack_pairs(a2d, dtype):
    k, f = a2d.shape
    j = k // (2 * _P)
    return np.ascontiguousarray(
        a2d.reshape(j, 2, _P, f).transpose(0, 2, 1, 3).reshape(j * _P, 2, f)
    ).astype(dtype)


def make_in_maps(x, W, centroids, b_loc=_B_LOC, n_cores=_NCORES):
    import ml_dtypes

    e4 = ml_dtypes.float8_e4m3

    x = np.asarray(x, dtype=np.float32)
    W = np.asarray(W, dtype=np.float32)
    centroids = np.asarray(centroids, dtype=np.float32)

    w_f16 = W.astype(np.float16)
    ct_p = _pack_pairs(np.ascontiguousarray(centroids.T), e4)
    xt_full = np.ascontiguousarray(x.T).astype(np.float16)

    maps = []
    for i in range(n_cores):
        xt_p = np.ascontiguousarray(xt_full[:, i * b_loc : (i + 1) * b_loc])
        maps.append({"xt": xt_p, "w": w_f16, "ct": ct_p})
    return maps


_NC_CACHE = {}


def kernel(x, W, centroids):
    from concourse.bass_utils import run_bass_kernel_spmd

    if "nc" not in _NC_CACHE:
        _NC_CACHE["nc"] = build_nc()
    nc = _NC_CACHE["nc"]

    in_maps = make_in_maps(x, W, centroids)
    res = run_bass_kernel_spmd(nc, in_maps, list(range(_NCORES)))
    return np.concatenate(
        [res.results[i]["out"].astype(np.float32) for i in range(_NCORES)], axis=0
    )


# revision 41
# speedup vs baseline: 1.3452x; 1.0040x over previous
"""Trainium2 Bass kernel for nn_CentroidModel (retrieval_knn).

HW-validated fallback (run 1): 281.8 us, rel err 1.256e-2.

Computes out = -(||e||^2 + ||c||^2 - 2 e.c) with e = x @ W, over 8
NeuronCores, data-parallel on the batch dim (x rows sharded; W and
centroids replicated).  Big GEMM in fp8e4 DoubleRow; fp16 phase-1;
fp16 output.
"""

import numpy as np

_B, _DIN, _D, _C = 8192, 1024, 768, 16384
_NCORES = 8
_B_LOC = _B // _NCORES

_P = 128
_NT = 512
_NW = 1024


def emit_centroid_kernel(tc, xt, w, ct, out, b_loc, din, d, c):
    from concourse import mybir
    from concourse.masks import make_identity

    nc = tc.nc
    e4 = mybir.dt.float8e4
    bf16 = mybir.dt.bfloat16
    f16 = mybir.dt.float16
    f32 = mybir.dt.float32
    AF = mybir.ActivationFunctionType
    DR = mybir.MatmulPerfMode.DoubleRow

    kd = din // _P
    jd = d // (2 * _P)
    md = d // _P
    mb = b_loc // _P
    npair = c // _NW

    with (
        tc.tile_pool(name="persist", bufs=1) as persist,
        tc.tile_pool(name="ct_in", bufs=9) as ct_pool,
        tc.tile_pool(name="sq", bufs=8) as sq_pool,
        tc.tile_pool(name="csum", bufs=2) as csum_pool,
        tc.tile_pool(name="csqs", bufs=3) as csq_pool,
        tc.tile_pool(name="t1", bufs=6) as t1_pool,
        tc.tile_pool(name="outs", bufs=10) as out_pool,
        tc.tile_pool(name="scratch", bufs=2) as scratch,
    ):
        xt_s = [persist.tile([_P, b_loc], f16, name=f"xt{k}", tag=f"xt{k}") for k in range(kd)]
        w_s = [persist.tile([_P, d], f16, name=f"w{k}", tag=f"w{k}") for k in range(kd)]
        et2_s = [persist.tile([_P, 2, b_loc], e4, name=f"et{j}", tag=f"et{j}") for j in range(jd)]
        negesq = persist.tile([_P, mb], f32, name="negesq", tag="negesq")
        ones = persist.tile([_P, _P], bf16, name="ones", tag="ones")
        ident = persist.tile([_P, _P], f32, name="ident", tag="ident")

        for k in range(kd):
            nc.sync.dma_start(xt_s[k][:], xt[k * _P : (k + 1) * _P, :])
            nc.sync.dma_start(w_s[k][:], w[k * _P : (k + 1) * _P, :])
        nc.vector.memset(ones[:], 1.0)
        make_identity(nc, ident[:])

        with tc.tile_pool(name="ps_pro", bufs=2, space="PSUM") as ps_pro:
            for m in range(md):
                pt = ps_pro.tile([_P, b_loc], f32, name="pro", tag="pro")
                for nb in range(b_loc // _NT):
                    bs = slice(nb * _NT, (nb + 1) * _NT)
                    for k in range(kd):
                        nc.tensor.matmul(
                            pt[:, bs],
                            w_s[k][:, m * _P : (m + 1) * _P],
                            xt_s[k][:, bs],
                            start=(k == 0),
                            stop=(k == kd - 1),
                        )
                nc.scalar.activation(
                    et2_s[m // 2][:, m % 2, :], pt[:], AF.Copy, scale=2.0
                )

            pesq = ps_pro.tile([_P, b_loc], f32, name="pesq", tag="pesq", bufs=1)
            for j in range(jd):
                sqe = scratch.tile([_P, 2, b_loc], bf16, name="sqe", tag="sqe")
                nc.vector.tensor_mul(sqe[:], et2_s[j][:], et2_s[j][:])
                for k2 in range(2):
                    for nb in range(b_loc // _NT):
                        bs = slice(nb * _NT, (nb + 1) * _NT)
                        nc.tensor.matmul(
                            pesq[:, bs],
                            ones[:],
                            sqe[:, k2, bs],
                            start=(j == 0 and k2 == 0),
                            stop=(j == jd - 1 and k2 == 1),
                        )
            esq_rep = scratch.tile([_P, b_loc], f32, name="esq_rep", tag="esq_rep")
            nc.scalar.activation(esq_rep[:], pesq[:], AF.Copy)
            for i in range(mb):
                ptr = ps_pro.tile([_P, _P], f32, name="ptr", tag="ptr")
                nc.tensor.transpose(ptr[:], esq_rep[:, i * _P : (i + 1) * _P], ident[:])
                nc.scalar.activation(negesq[:, i : i + 1], ptr[:, 0:1], AF.Copy, scale=-0.25)

        with (
            tc.tile_pool(name="ps_big", bufs=3, space="PSUM") as ps_big,
            tc.tile_pool(name="ps_csq", bufs=1, space="PSUM") as ps_csq,
        ):
            def load_ct(n):
                csl = slice(n * _NW, (n + 1) * _NW)
                tiles = []
                for j in range(jd):
                    t = ct_pool.tile([_P, 2, _NW], e4, name=f"ct{j}", tag="ct")
                    nc.sync.dma_start(t[:], ct[j * _P : (j + 1) * _P, :, csl])
                    tiles.append(t)
                return tiles

            def csq_front(ct_t):
                # DVE square/add chain — emitted as soon as the chunk's ct
                # lands so it runs under the previous chunk's matmuls
                sqs = []
                for j in range(jd):
                    sq_t = sq_pool.tile([_P, 2, _NW], bf16, name="sqc", tag="sqc")
                    nc.vector.tensor_mul(sq_t[:], ct_t[j][:], ct_t[j][:])
                    sqs.append(sq_t)
                nc.vector.tensor_add(sqs[0][:], sqs[0][:], sqs[1][:])
                nc.vector.tensor_add(sqs[0][:], sqs[0][:], sqs[2][:])
                csum = csum_pool.tile([_P, _NW], bf16, name="csum", tag="csum")
                nc.vector.tensor_add(csum[:], sqs[0][:, 0, :], sqs[0][:, 1, :])
                return csum

            def csq_back(csum):
                # PE ones-matmuls + ACT evac — emitted LATE (i == mb-2) so
                # they never head-of-line block the in-order PE queue while
                # the DVE chain finishes (~2.7 us/chunk measured)
                pcs = ps_csq.tile([_P, _NW], f32, name="csq", tag="csq")
                csq_s = csq_pool.tile([_P, _NW], f16, name="csq_s", tag="csq_s")
                for h in range(2):
                    hs = slice(h * _NT, (h + 1) * _NT)
                    nc.tensor.matmul(pcs[:, hs], ones[:], csum[:, hs], start=True, stop=True)
                nc.scalar.activation(csq_s[:], pcs[:], AF.Copy)
                return csq_s

            pending_stores = []
            ct_cur = load_ct(0)
            csq_cur = csq_back(csq_front(ct_cur))
            for n in range(npair):
                csl = slice(n * _NW, (n + 1) * _NW)
                ct_nxt = load_ct(n + 1) if n + 1 < npair else None
                csum_nxt = csq_front(ct_nxt) if ct_nxt is not None else None
                for dst, src_t in pending_stores:
                    nc.sync.dma_start(dst, src_t[:])
                pending_stores = []

                for i in range(mb):
                    pb = ps_big.tile([_P, _NW], f32, name="big", tag="big")
                    for j in range(jd):
                        lhsT = et2_s[j][:, :, i * _P : (i + 1) * _P]
                        nc.tensor.matmul(
                            pb[:, 0:_NT], lhsT, ct_cur[j][:, :, 0:_NT],
                            start=(j == 0), stop=(j == jd - 1), perf_mode=DR,
                        )
                        nc.tensor.matmul(
                            pb[:, _NT:_NW], lhsT, ct_cur[j][:, :, _NT:_NW],
                            start=(j == 0), stop=(j == jd - 1), perf_mode=DR,
                        )
                    t1 = t1_pool.tile([_P, _NW], f16, name="t1", tag="t1")
                    nc.scalar.activation(
                        t1[:], pb[:], AF.Identity, bias=negesq[:, i : i + 1]
                    )
                    ot = out_pool.tile([_P, _NW], f16, name="ot", tag="ot")
                    nc.vector.tensor_sub(ot[:], t1[:], csq_cur[:])
                    if n == npair - 1:
                        nc.sync.dma_start(out[i * _P : (i + 1) * _P, csl], ot[:])
                    else:
                        pending_stores.append((out[i * _P : (i + 1) * _P, csl], ot))
                    if i == mb - 2 and csum_nxt is not None:
                        csq_nxt = csq_back(csum_nxt)
                if ct_nxt is not None:
                    ct_cur, csq_cur = ct_nxt, csq_nxt
            for dst, src_t in pending_stores:
                nc.sync.dma_start(dst, src_t[:])


def build_nc(b_loc=_B_LOC, din=_DIN, d=_D, c=_C):
    import concourse.tile as tile
    from concourse import bacc, mybir

    nc = bacc.Bacc("TRN2", target_bir_lowering=False, debug=False)
    jd = d // (2 * _P)
    xt = nc.declare_dram_parameter("xt", [din, b_loc], mybir.dt.float16, isOutput=False)
    w = nc.declare_dram_parameter("w", [din, d], mybir.dt.float16, isOutput=False)
    ct = nc.declare_dram_parameter("ct", [jd * _P, 2, c], mybir.dt.float8e4, isOutput=False)
    out = nc.declare_dram_parameter("out", [b_loc, c], mybir.dt.float16, isOutput=True)
    with tile.TileContext(nc) as tc:
        emit_centroid_kernel(tc, xt.ap(), w.ap(), ct.ap(), out.ap(), b_loc, din, d, c)
    nc.compile()
    return nc


def _pack_pairs(a2d, dtype):
    k, f = a2d.shape
    j = k // (2 * _P)
    return np.ascontiguousarray(
        a2d.reshape(j, 2, _P, f).transpose(0, 2, 1, 3).reshape(j * _P, 2, f)
    ).astype(dtype)


def make_in_maps(x, W, centroids, b_loc=_B_LOC, n_cores=_NCORES):
    import ml_dtypes

    e4 = ml_dtypes.float8_e4m3

    x = np.asarray(x, dtype=np.float32)
    W = np.asarray(W, dtype=np.float32)
    centroids = np.asarray(centroids, dtype=np.float32)

    w_f16 = W.astype(np.float16)
    ct_p = _pack_pairs(np.ascontiguousarray(centroids.T), e4)
    xt_full = np.ascontiguousarray(x.T).astype(np.float16)

    maps = []
    for i in range(n_cores):
        xt_p = np.ascontiguousarray(xt_full[:, i * b_loc : (i + 1) * b_loc])
        maps.append({"xt": xt_p, "w": w_f16, "ct": ct_p})
    return maps


_NC_CACHE = {}


def kernel(x, W, centroids):
    from concourse.bass_utils import run_bass_kernel_spmd

    if "nc" not in _NC_CACHE:
        _NC_CACHE["nc"] = build_nc()
    nc = _NC_CACHE["nc"]

    in_maps = make_in_maps(x, W, centroids)
    res = run_bass_kernel_spmd(nc, in_maps, list(range(_NCORES)))
    return np.concatenate(
        [res.results[i]["out"].astype(np.float32) for i in range(_NCORES)], axis=0
    )
